# revision 1
# baseline (speedup 1.0000x reference)
"""JointRetention Trainium2 kernel.

out[b] = ((xpos(X_b Wq) xpos_down(X_b Wk)^T) * D[b%17]) @ (X_b Wv)

Strategy:
  - Data-parallel over B*J=1088 across 8 cores (136 each; 136%17==0 so the
    joint index pattern is identical on every core).
  - rotate_every_two folded into host-precomputed W@R so xpos becomes
    Qx = (X Wq) * C + (X Wq R) * S  -- two matmuls + elementwise.
  - All tensors kept transposed on-chip (head dim on partitions) so
    S^T = Kx^T-major matmul feeds the masked A^T @ V matmul directly.
  - float32r matmuls (1 cycle/row when N>=256), batch pairs packed into
    512-wide tiles so every matmul free dim is 256/512.
"""

import numpy as np

L = 243
H = 256
J = 17
NCORES = 8
NB = 1088
BPC = NB // NCORES          # 136 batch rows per core
NPAIR = BPC // 2            # 68 pairs per core
SCALE_BASE = 512
CHUNK = 81

_cache = {}


def _host_tables(W_Q, W_K, W_V, gamma):
    f32 = np.float32
    # rot(y) = y @ R
    R = np.zeros((H, H), f32)
    idx = np.arange(0, H, 2)
    R[idx + 1, idx] = -1.0
    R[idx, idx + 1] = 1.0

    WQ = W_Q.astype(f32)
    WK = W_K.astype(f32)
    WV = W_V.astype(f32)
    WQR = (WQ @ R).astype(f32)
    WKR = (WK @ R).astype(f32)

    # xpos coefficient tables (L, H) then transposed to (H, L)
    half = H // 2
    base_scale = ((np.arange(0, H, 2, dtype=f32) + 0.4 * H) / (1.4 * H)).astype(f32)
    pos = np.arange(L, dtype=f32)
    scale = base_scale[None, :] ** (pos / SCALE_BASE)[:, None]        # (L, half)
    inv_freq = (1.0 / 10000.0 ** (np.arange(half, dtype=f32) / half)).astype(f32)
    sinus = pos[:, None] * inv_freq[None, :]
    sin, cos = np.sin(sinus).astype(f32), np.cos(sinus).astype(f32)

    def dup(m):
        return np.repeat(m, 2, axis=-1)

    CQ = dup(cos * scale).T.astype(f32)      # (H, L)
    SQ = dup(sin * scale).T.astype(f32)
    inv = (1.0 / scale).astype(f32)
    CK = dup(cos * inv).T.astype(f32)
    SK = dup(sin * inv).T.astype(f32)

    # pack weights: per h-chunk rows, cols [WQ | WQR | WK | WKR | WV] (1280)
    Wcat = np.concatenate([WQ, WQR, WK, WKR, WV], axis=1)            # (256, 1280)
    W_all = np.stack([Wcat[0:128], Wcat[128:256]], axis=0)           # (2, 128, 1280)

    # pack tables: per d-chunk rows, cols [CQ | SQ | CK | SK] each 512 wide
    # (pair layout: cols 0:243 = b0, 256:499 = b1, pads zero)
    def pack(tbl, dc):
        out = np.zeros((128, 512), f32)
        rows = tbl[dc * 128:(dc + 1) * 128]
        out[:, 0:L] = rows
        out[:, 256:256 + L] = rows
        return out

    CS = np.zeros((2, 128, 2048), f32)
    for dc in range(2):
        CS[dc, :, 0:512] = pack(CQ, dc)
        CS[dc, :, 512:1024] = pack(SQ, dc)
        CS[dc, :, 1024:1536] = pack(CK, dc)
        CS[dc, :, 1536:2048] = pack(SK, dc)

    # decay mask, transposed per joint, free dim padded to 256
    g = gamma.astype(f32)
    i = np.arange(L)[:, None]
    jj = np.arange(L)[None, :]
    allowed = jj < (i // CHUNK + 1) * CHUNK
    absd = np.abs(i - jj).astype(f32)
    D = g[:, None, None] ** absd[None]                               # (J, L, L)
    D = np.where(allowed[None], D, 0.0)
    D = np.where(np.isnan(D), 0.0, D).astype(f32)
    DT = np.zeros((J, L, 256), f32)
    DT[:, :, 0:L] = np.transpose(D, (0, 2, 1))                       # DT[j, m, l]

    ident = np.eye(128, dtype=f32)
    return W_all, CS, DT, ident


def _build():
    import concourse.bacc as bacc
    import concourse.mybir as mybir
    from concourse import tile

    dt = mybir.dt
    f32 = dt.float32
    f32r = dt.float32r

    nc = bacc.Bacc("TRN2", target_bir_lowering=False, debug=False,
                   num_devices=NCORES)
    X_d = nc.dram_tensor("X", (BPC, L, H), f32, kind="ExternalInput").ap()
    W_d = nc.dram_tensor("WALL", (2, 128, 1280), f32, kind="ExternalInput").ap()
    CS_d = nc.dram_tensor("CS", (2, 128, 2048), f32, kind="ExternalInput").ap()
    DT_d = nc.dram_tensor("DTAB", (J, L, 256), f32, kind="ExternalInput").ap()
    ID_d = nc.dram_tensor("IDEN", (128, 128), f32, kind="ExternalInput").ap()
    O_d = nc.dram_tensor("OUT", (BPC, L, H), f32, kind="ExternalOutput").ap()

    def rr(ap):
        return ap.bitcast(f32r)

    LSZ = (128, L - 128)          # l/m chunk sizes (128, 115)

    with tile.TileContext(nc) as tc:
        with (
            tc.tile_pool(name="const", bufs=1) as const,
            tc.tile_pool(name="xin", bufs=3) as xin,
            tc.tile_pool(name="work", bufs=2) as work,
            tc.tile_pool(name="pxt", bufs=2, space="PSUM") as pxt,
            tc.tile_pool(name="pproj", bufs=3, space="PSUM") as pproj,
            tc.tile_pool(name="pv", bufs=1, space="PSUM") as pv,
            tc.tile_pool(name="pso", bufs=2, space="PSUM") as pso,
        ):
            # ---- constants ----
            w_sb = [const.tile([128, 1280], f32, name=f"w{h}", tag=f"w{h}") for h in range(2)]
            cs_sb = [const.tile([128, 2048], f32, name=f"cs{d}", tag=f"cs{d}") for d in range(2)]
            ident = const.tile([128, 128], f32, name="ident", tag="ident")
            dt_sb = [[const.tile([LSZ[mc], 256], f32, name=f"dt{j}_{mc}", tag=f"dt{j}_{mc}")
                      for mc in range(2)] for j in range(J)]
            w_r = [const.tile([128, 1280], f32r, name=f"wr{h}", tag=f"wr{h}")
                   for h in range(2)]
            for h in range(2):
                nc.sync.dma_start(w_sb[h][:], W_d[h])
                nc.sync.dma_start(cs_sb[h][:], CS_d[h])
                nc.scalar.copy(w_r[h][:], w_sb[h][:])
            nc.sync.dma_start(ident[:], ID_d[:])
            for j in range(J):
                for mc in range(2):
                    nc.sync.dma_start(dt_sb[j][mc][:],
                                      DT_d[j, mc * 128:mc * 128 + LSZ[mc], :])

            for t in range(NPAIR):
                b0 = 2 * t
                joints = (b0 % J, (b0 + 1) % J)

                # ---- load X pair ----
                xt_in = []
                for k in range(2):
                    row = []
                    for lc in range(2):
                        tl = xin.tile([LSZ[lc], H], f32, name=f"x{k}{lc}", tag=f"x{k}{lc}")
                        nc.sync.dma_start(
                            tl[:], X_d[b0 + k, lc * 128:lc * 128 + LSZ[lc], :])
                        row.append(tl)
                    xt_in.append(row)

                # ---- transpose X -> XT (h on partitions), pair packed ----
                # psum cols: b0 at 0:243, b1 at 243:486
                xt_sb = []
                for h in range(2):
                    ps = pxt.tile([128, 512], f32, name="xtp", tag="xtp")
                    for k in range(2):
                        for lc in range(2):
                            col = k * L + lc * 128
                            nc.tensor.transpose(
                                ps[:, col:col + LSZ[lc]],
                                xt_in[k][lc][:, h * 128:(h + 1) * 128],
                                ident[0:LSZ[lc], 0:LSZ[lc]],
                            )
                    sb = work.tile([128, 512], f32r, name=f"xt{h}", tag=f"xt{h}")
                    # repack: b0 -> 0:243, b1 -> 256:499 (pads never read as
                    # real data; CS tables carry zeros in pad cols)
                    nc.scalar.copy(sb[:, 0:L], ps[:, 0:L])
                    nc.scalar.copy(sb[:, 256:256 + L], ps[:, L:2 * L])
                    xt_sb.append(sb)

                # ---- V = X @ Wv  (natural layout: l on partitions) ----
                v_sb = []
                for lc in range(2):
                    ps = pv.tile([128, 512], f32, name="vp", tag="vp")
                    for k in range(2):
                        for h in range(2):
                            nc.tensor.matmul(
                                ps[0:LSZ[lc], k * 256:k * 256 + 256],
                                xt_sb[h][:, k * 256 + lc * 128:
                                        k * 256 + lc * 128 + LSZ[lc]],
                                w_r[h][:, 1024:1280],
                                start=(h == 0), stop=(h == 1),
                            )
                    sb = work.tile([128, 512], f32r, name=f"v{lc}", tag=f"v{lc}")
                    nc.scalar.copy(sb[0:LSZ[lc], :], ps[0:LSZ[lc], :])
                    v_sb.append(sb)

                # ---- projections (transposed: d on partitions) + xpos ----
                # tensors: 0=Q, 1=QR, 2=K, 3=KR ; combine pairs (0,1)->Qx, (2,3)->Kx
                qx, kx = [], []
                for pair_i, dst in ((0, qx), (2, kx)):
                    for dc in range(2):
                        ps_a = pproj.tile([128, 512], f32, name="proj", tag="proj")
                        ps_b = pproj.tile([128, 512], f32, name="proj", tag="proj")
                        for h in range(2):
                            nc.tensor.matmul(
                                ps_a[:],
                                w_r[h][:, pair_i * 256 + dc * 128:
                                       pair_i * 256 + dc * 128 + 128],
                                xt_sb[h][:],
                                start=(h == 0), stop=(h == 1),
                            )
                        for h in range(2):
                            nc.tensor.matmul(
                                ps_b[:],
                                w_r[h][:, (pair_i + 1) * 256 + dc * 128:
                                       (pair_i + 1) * 256 + dc * 128 + 128],
                                xt_sb[h][:],
                                start=(h == 0), stop=(h == 1),
                            )
                        # xpos: out = ps_a * C + ps_b * S
                        cbase = (0 if pair_i == 0 else 1024)
                        t1 = work.tile([128, 512], f32, name="t1", tag="t1")
                        t2 = work.tile([128, 512], f32, name="t2", tag="t2")
                        nc.vector.tensor_mul(
                            t1[:], ps_a[:], cs_sb[dc][:, cbase:cbase + 512])
                        nc.vector.tensor_mul(
                            t2[:], ps_b[:], cs_sb[dc][:, cbase + 512:cbase + 1024])
                        out = work.tile([128, 512], f32r,
                                        name=f"{'qx' if pair_i == 0 else 'kx'}{dc}",
                                        tag=f"{'qx' if pair_i == 0 else 'kx'}{dc}")
                        nc.gpsimd.tensor_add(out[:], t1[:], t2[:])
                        dst.append(out)

                # ---- attention per batch element ----
                for k in range(2):
                    jt = joints[k]
                    at = []
                    for mc in range(2):
                        msz = LSZ[mc]
                        ps = pso.tile([128, 256], f32, name="so", tag="so")
                        for dc in range(2):
                            nc.tensor.matmul(
                                ps[0:msz, :],
                                kx[dc][:, k * 256 + mc * 128:
                                       k * 256 + mc * 128 + msz],
                                qx[dc][:, k * 256:k * 256 + 256],
                                start=(dc == 0), stop=(dc == 1),
                            )
                        a = work.tile([LSZ[mc], 256], f32r, name=f"at{mc}", tag=f"at{mc}")
                        nc.vector.tensor_mul(a[:], ps[0:msz, :], dt_sb[jt][mc][:])
                        at.append(a)
                    for lc in range(2):
                        lsz = LSZ[lc]
                        ps = pso.tile([128, 256], f32, name="so", tag="so")
                        for mc in range(2):
                            nc.tensor.matmul(
                                ps[0:lsz, :],
                                at[mc][:, lc * 128:lc * 128 + lsz],
                                v_sb[mc][0:LSZ[mc], k * 256:k * 256 + 256],
                                start=(mc == 0), stop=(mc == 1),
                            )
                        ob = work.tile([128, 256], f32, name=f"ob{lc}",
                                       tag=f"ob{lc}")
                        nc.scalar.copy(ob[0:lsz, :], ps[0:lsz, :])
                        nc.sync.dma_start(
                            O_d[b0 + k, lc * 128:lc * 128 + lsz, :],
                            ob[0:lsz, :])

    nc.compile()
    return nc


def _get_nc():
    if "nc" not in _cache:
        _cache["nc"] = _build()
    return _cache["nc"]


def _run(in_maps, trace=False):
    from concourse import bass_utils
    nc = _get_nc()
    return bass_utils.run_bass_kernel_spmd(
        nc, in_maps, core_ids=list(range(NCORES)), trace=trace)


def kernel(X, W_Q, W_K, W_V, gamma, _trace=False):
    X = np.asarray(X, np.float32)
    W_all, CS, DT, ident = _host_tables(
        np.asarray(W_Q, np.float32), np.asarray(W_K, np.float32),
        np.asarray(W_V, np.float32), np.asarray(gamma, np.float32))

    in_maps = []
    for c in range(NCORES):
        in_maps.append({
            "X": np.ascontiguousarray(X[c * BPC:(c + 1) * BPC]),
            "WALL": W_all, "CS": CS, "DTAB": DT, "IDEN": ident,
        })
    res = _run(in_maps, trace=_trace)
    out = np.concatenate([r["OUT"] for r in res.results], axis=0)
    if _trace:
        _cache["last_result"] = res
    return out

